# revision 7
# baseline (speedup 1.0000x reference)
"""MCSPN Trainium2 kernel: guidance convs + softmax gates + 4-step CSPN recurrence.

Data-parallel over batch: 8 images -> 8 NeuronCores, one image per core.
Per core:
  phase A: conv3x3 (fp32r matmuls, 18 accum MMs/row) -> bias+ReLU (ACT)
           -> conv1x1 (fp32r) -> exp (ACT) -> per-row DMA scatter into
           gate layout e_all [H=128 part, 76*256 free]
  softmax: 3 adds + reciprocal + 4 muls over [128, 19*256] strided views
  phase B: 4 recurrence steps; left/right via guarded 258-wide windows of h,
           up/down via PE shift-matmuls (sub/super-diagonal fp32r matrices)
           into PSUM; gated sums on DVE + GPSIMD.
"""
import os
import sys

sys.path.insert(0, "/opt/trn_rl_repo")

import numpy as np

B, CIN, H, W = 8, 256, 128, 256
K = 19
MID = 128
KD = 4 * K  # 76
EPS = 1e-5
T_STEPS = 4
WP = W + 2  # guarded row width (258)
RG = 8      # feats rows per DMA chunk


def _build():
    import concourse.bacc as bacc
    import concourse.mybir as mybir
    import concourse.tile as tile
    from concourse import bass

    f32 = mybir.dt.float32
    f32r = mybir.dt.float32r
    Act = mybir.ActivationFunctionType
    Alu = mybir.AluOpType

    nc = bacc.Bacc("TRN2", target_bir_lowering=False)

    feats_d = nc.dram_tensor("feats", [CIN, H, W], f32, kind="ExternalInput")
    logits_d = nc.dram_tensor("logits", [K, H, W], f32, kind="ExternalInput")
    w1t_d = nc.dram_tensor("w1t", [128, 2, 9, MID], f32, kind="ExternalInput")
    bmid_d = nc.dram_tensor("bmid", [MID, 1], f32, kind="ExternalInput")
    w2t_d = nc.dram_tensor("w2t", [MID, KD], f32, kind="ExternalInput")
    b2_d = nc.dram_tensor("b2", [KD, 1], f32, kind="ExternalInput")
    sup_d = nc.dram_tensor("sup", [128, 128], f32, kind="ExternalInput")
    sdn_d = nc.dram_tensor("sdn", [128, 128], f32, kind="ExternalInput")
    out_d = nc.dram_tensor("out", [K, H, W], f32, kind="ExternalOutput")

    with tile.TileContext(nc) as tc:
        # ---- long-lived tensors ----
        with tc.tile_pool(name="persist", bufs=1) as pp, \
             tc.tile_pool(name="hpool", bufs=1) as hp:
            e_all = pp.tile([128, KD * W], f32)           # 76 KB/part
            h_a = hp.tile([128, K * WP], f32r)            # 19.6 KB/part
            h_b = hp.tile([128, K * WP], f32r)
            w2_r = pp.tile([MID, KD], f32r)
            bmid = pp.tile([MID, 1], f32)
            b2c = pp.tile([KD, 1], f32)
            s_up = pp.tile([128, 128], f32r)
            s_dn = pp.tile([128, 128], f32r)
            z32 = pp.tile([128, 64], f32)  # zeros source for f32r guard writes

            nc.vector.memset(z32[:], 0.0)
            nc.sync.dma_start(out=bmid[:], in_=bmid_d[:])
            nc.sync.dma_start(out=b2c[:], in_=b2_d[:])
            with tc.tile_pool(name="stage", bufs=1) as stp:
                w2_f = stp.tile([MID, KD], f32)
                s_up_f = stp.tile([128, 128], f32)
                s_dn_f = stp.tile([128, 128], f32)
                nc.sync.dma_start(out=w2_f[:], in_=w2t_d[:])
                nc.vector.tensor_copy(out=w2_r[:], in_=w2_f[:])
                nc.sync.dma_start(out=s_up_f[:], in_=sup_d[:])
                nc.vector.tensor_copy(out=s_up[:], in_=s_up_f[:])
                nc.sync.dma_start(out=s_dn_f[:], in_=sdn_d[:])
                nc.vector.tensor_copy(out=s_dn[:], in_=s_dn_f[:])

            # ================= phase A: guidance =================
            with tc.tile_pool(name="w1p", bufs=1) as w1p:
                w1_f = w1p.tile([128, 2, 9, MID], f32)
                w1_r = w1p.tile([128, 2, 9, MID], f32r)
                nc.sync.dma_start(out=w1_f[:], in_=w1t_d[:])
                nc.vector.tensor_copy(out=w1_r[:], in_=w1_f[:])

                with tc.tile_pool(name="frows", bufs=4) as frp, \
                     tc.tile_pool(name="xrow", bufs=3) as xrp, \
                     tc.tile_pool(name="estrip", bufs=3) as esp, \
                     tc.tile_pool(name="psA", bufs=3, space="PSUM") as psA, \
                     tc.tile_pool(name="psG", bufs=3, space="PSUM") as psG:
                    n_groups = H // RG
                    ftiles = []  # group idx -> tile [128, 2, RG, WP]
                    for gi in range(n_groups):
                        ft = frp.tile([128, 2, RG, WP], f32r, name=f"ft{gi}",
                                      tag="ft")
                        # zero guard columns (both chunks, all rows) via
                        # rounding copy (memset can't write f32r)
                        nc.vector.tensor_copy(
                            out=ft[:, :, :, 0:WP:WP - 1],
                            in_=z32[:, 0:32].rearrange(
                                "p (a b c) -> p a b c", a=2, b=RG))
                        for c in range(2):
                            nc.sync.dma_start(
                                out=ft[:, c, :, 1:W + 1],
                                in_=feats_d[c * 128:(c + 1) * 128,
                                            gi * RG:(gi + 1) * RG, :]
                                .bitcast(f32r))
                        ftiles.append(ft)

                        # rows of group gi-1 are fully computable once group gi
                        # is loaded (need halo row below); process them now.
                        lo = 0 if gi == 0 else (gi - 1) * RG
                        hi = (gi + 1) * RG - 1 if gi == n_groups - 1 else gi * RG - 1
                        for y in range(lo, hi + 1):
                            acc = psA.tile([MID, W], f32, name="acc")
                            taps = [(ky, c, kx)
                                    for ky in range(3)
                                    if 0 <= y + ky - 1 < H
                                    for c in range(2)
                                    for kx in range(3)]
                            for i, (ky, c, kx) in enumerate(taps):
                                ys = y + ky - 1
                                src = ftiles[ys // RG]
                                nc.tensor.matmul(
                                    out=acc[:],
                                    lhsT=w1_r[:, c, ky * 3 + kx, :],
                                    rhs=src[:, c, ys % RG, kx:kx + W],
                                    start=(i == 0), stop=(i == len(taps) - 1))
                            # relu(x + bias) -> f32r
                            xr = xrp.tile([MID, W], f32r, name="xr")
                            nc.scalar.activation(xr[:], acc[:], Act.Relu,
                                                 bias=bmid[:], scale=1.0)
                            accg = psG.tile([KD, W], f32, name="accg")
                            nc.tensor.matmul(out=accg[:], lhsT=w2_r[:],
                                             rhs=xr[:], start=True, stop=True)
                            # exp(g + b2) -> strip then scatter to e_all row y
                            es = esp.tile([KD, W], f32, name="es")
                            nc.scalar.activation(es[:], accg[:], Act.Exp,
                                                 bias=b2c[:], scale=1.0)
                            nc.sync.dma_start(
                                out=e_all[y:y + 1, :].rearrange(
                                    "p (c w) -> p c w", c=KD),
                                in_=es[:])

            # ================= softmax over 4 directions =================
            with tc.tile_pool(name="smx", bufs=1) as sp:
                s_all = sp.tile([128, K * W], f32)
                r_all = sp.tile([128, K * W], f32)
                ev = e_all[:].rearrange("p (k d w) -> p k d w", k=K, d=4)
                sv = s_all[:].rearrange("p (k w) -> p k w", k=K)
                nc.vector.tensor_tensor(out=sv, in0=ev[:, :, 0, :],
                                        in1=ev[:, :, 1, :], op=Alu.add)
                nc.vector.tensor_tensor(out=sv, in0=sv,
                                        in1=ev[:, :, 2, :], op=Alu.add)
                nc.vector.tensor_tensor(out=sv, in0=sv,
                                        in1=ev[:, :, 3, :], op=Alu.add)
                rv = r_all[:].rearrange("p (k w) -> p k w", k=K)
                nc.vector.reciprocal(out=r_all[:], in_=s_all[:])
                for d in range(4):
                    eng = nc.vector if d % 2 == 0 else nc.gpsimd
                    eng.tensor_tensor(out=ev[:, :, d, :], in0=ev[:, :, d, :],
                                      in1=rv, op=Alu.mult)

            # ---- load h0 = logits into guarded layout ----
            hv_a = h_a[:].rearrange("p (k w) -> p k w", k=K)
            hv_b = h_b[:].rearrange("p (k w) -> p k w", k=K)
            nc.vector.tensor_copy(
                out=hv_a[:, :, 0:WP:WP - 1],
                in_=z32[:, 0:2 * K].rearrange("p (k g) -> p k g", k=K))
            nc.vector.tensor_copy(
                out=hv_b[:, :, 0:WP:WP - 1],
                in_=z32[:, 0:2 * K].rearrange("p (k g) -> p k g", k=K))
            for k in range(K):
                nc.sync.dma_start(
                    out=h_a[:, k * WP + 1:k * WP + 1 + W],
                    in_=logits_d[k].bitcast(f32r))

            # ================= phase B: recurrence =================
            if True:
                with tc.tile_pool(name="tmp", bufs=4) as tp, \
                     tc.tile_pool(name="psS", bufs=3, space="PSUM") as psS:
                    cur, nxt = h_a, h_b
                    for t in range(T_STEPS):
                        for k in range(K):
                            base = k * WP
                            hwin = cur[:, base:base + WP]
                            up_ps = psS.tile([128, WP], f32, name="up_ps")
                            dn_ps = psS.tile([128, WP], f32, name="dn_ps")
                            nc.tensor.matmul(out=up_ps[:], lhsT=s_up[:],
                                             rhs=hwin, start=True, stop=True)
                            nc.tensor.matmul(out=dn_ps[:], lhsT=s_dn[:],
                                             rhs=hwin, start=True, stop=True)
                            gl = e_all[:, (4 * k + 0) * W:(4 * k + 1) * W]
                            gr = e_all[:, (4 * k + 1) * W:(4 * k + 2) * W]
                            gu = e_all[:, (4 * k + 2) * W:(4 * k + 3) * W]
                            gd = e_all[:, (4 * k + 3) * W:(4 * k + 4) * W]
                            left = cur[:, base:base + W].bitcast(f32)
                            right = cur[:, base + 2:base + 2 + W].bitcast(f32)
                            a = tp.tile([128, W], f32, name="a")
                            b = tp.tile([128, W], f32, name="b")
                            c2 = tp.tile([128, W], f32, name="c2")
                            d2 = tp.tile([128, W], f32, name="d2")
                            nc.vector.tensor_tensor(out=a[:], in0=gl, in1=left,
                                                    op=Alu.mult)
                            nc.gpsimd.tensor_tensor(out=b[:], in0=gr, in1=right,
                                                    op=Alu.mult)
                            nc.vector.tensor_tensor(out=c2[:], in0=gu,
                                                    in1=up_ps[:, 1:W + 1],
                                                    op=Alu.mult)
                            nc.vector.tensor_tensor(out=d2[:], in0=gd,
                                                    in1=dn_ps[:, 1:W + 1],
                                                    op=Alu.mult)
                            nc.gpsimd.tensor_tensor(out=a[:], in0=a[:], in1=b[:],
                                                    op=Alu.add)
                            nc.vector.tensor_tensor(out=c2[:], in0=c2[:],
                                                    in1=d2[:], op=Alu.add)
                            nc.vector.tensor_tensor(
                                out=nxt[:, base + 1:base + 1 + W],
                                in0=a[:], in1=c2[:], op=Alu.add)
                        cur, nxt = nxt, cur

                    for k in range(K):
                        nc.sync.dma_start(
                            out=out_d[k],
                            in_=cur[:, k * WP + 1:k * WP + 1 + W].bitcast(f32))

    nc.compile()
    return nc


_NC_CACHE = None


def kernel(feats, logits, w1, gamma, beta, mean, var, w2, b2):
    global _NC_CACHE
    from concourse.bass_utils import run_bass_kernel_spmd

    feats = np.asarray(feats, dtype=np.float32)
    logits = np.asarray(logits, dtype=np.float32)
    w1 = np.asarray(w1, dtype=np.float32)
    w2 = np.asarray(w2, dtype=np.float32)
    b2 = np.asarray(b2, dtype=np.float32)
    gamma = np.asarray(gamma, dtype=np.float32)
    beta = np.asarray(beta, dtype=np.float32)
    mean = np.asarray(mean, dtype=np.float32)
    var = np.asarray(var, dtype=np.float32)

    inv = gamma / np.sqrt(var + EPS)
    w1f = (w1 * inv[:, None, None, None]).astype(np.float32)  # [MID,CIN,3,3]
    bmid = (beta - mean * inv).astype(np.float32)[:, None]    # [MID,1]
    # [cin_in_chunk 128, chunk 2, tap 9, mid 128]
    w1t = (w1f.transpose(1, 2, 3, 0)                  # [CIN,3,3,MID]
           .reshape(2, 128, 9, MID)
           .transpose(1, 0, 2, 3)).copy()
    w2t = w2.reshape(KD, MID).T.copy()                # [MID,KD]
    b2c = b2[:, None].copy()
    s_up = np.eye(128, k=1, dtype=np.float32)         # out[m]=h[m-1]
    s_dn = np.eye(128, k=-1, dtype=np.float32)        # out[m]=h[m+1]

    if _NC_CACHE is None:
        _NC_CACHE = _build()
    nc = _NC_CACHE

    in_maps = []
    for i in range(B):
        in_maps.append({
            "feats": np.ascontiguousarray(feats[i]),
            "logits": np.ascontiguousarray(logits[i]),
            "w1t": w1t, "bmid": bmid, "w2t": w2t, "b2": b2c,
            "sup": s_up, "sdn": s_dn,
        })

    trace = bool(os.environ.get("KTRACE"))
    res = run_bass_kernel_spmd(nc, in_maps, list(range(B)), trace=trace)
    if trace and res.exec_time_ns is not None:
        print(f"HW exec time: {res.exec_time_ns} ns")
    out = np.stack([res.results[i]["out"] for i in range(B)], axis=0)
    return out.astype(np.float32)


if __name__ == "__main__":
    rng = np.random.default_rng(0)
    ins = {
        "feats": rng.standard_normal((B, CIN, H, W), dtype=np.float32),
        "logits": rng.standard_normal((B, K, H, W), dtype=np.float32),
        "w1": rng.standard_normal((MID, CIN, 3, 3), dtype=np.float32) / 48.0,
        "gamma": rng.standard_normal(MID).astype(np.float32) * 0.1 + 1.0,
        "beta": rng.standard_normal(MID).astype(np.float32) * 0.1,
        "mean": rng.standard_normal(MID).astype(np.float32) * 0.1,
        "var": rng.random(MID).astype(np.float32) + 0.5,
        "w2": rng.standard_normal((KD, MID, 1, 1)).astype(np.float32) / 11.3,
        "b2": rng.standard_normal(KD).astype(np.float32) * 0.01,
    }
    o = kernel(**ins)
    print("kernel out", o.shape, o.dtype, np.abs(o).mean())
